# revision 18
# baseline (speedup 1.0000x reference)
"""Trainium2 Bass kernel for BCE + distance-decorrelation (DisCo) loss.

Reference math (N = 8192):
    bce  = mean((softplus(o) - o*l) * w)
    nw   = w * N / sum(w)
    a_ij = |o_i - o_j|, b_ij = |e_i - e_j|
    u_i  = (1/N) sum_j a_ij nw_j          (amatavg)
    A    = a - u_j - u_i + mA,  mA = (1/N) sum nw u    (same for B with v, mB)
    num  = (1/N^2) sum_ij nw_i nw_j A_ij B_ij
    den  = [(1/N^2) sum nw nw A^2] [(1/N^2) sum nw nw B^2]
    disco = num / sqrt(den);  tot = bce + 0.1 * disco

Exact algebraic decomposition (for arbitrary device weights omega, S = sum omega):
    num*N^2  = T_ab + (2S-4N) P_uv + (4N^2-4NS+S^2) mA mB
    denA*N^2 = T_aa + (2S-4N) P_uu + (4N^2-4NS+S^2) mA^2
    T_aa     = 2 S sum(om o^2) - 2 (sum om o)^2          (closed form, O(N))
    T_ab     = sum_i om_i t_i,  t_i = sum_j om_j a_ij b_ij
so the only O(N^2) device work is three per-row weighted sums:
    su_i = sum_j om_j a_ij, sv_i = sum_j om_j b_ij, st_i = sum_j om_j a_ij b_ij.

Sharding: core c owns rows [c*1024, (c+1)*1024). Tiles are laid out
[j (partition, 128 per block, 64 blocks) x i (free, 1024 rows)]:
    a = Abs(o_bcast + (-o_j))            on ScalarE (activation, per-partition bias)
    b = abs_max(e_bcast + (-e_j), 0)     on VectorE (one tensor_scalar op)
    ab = a * b                           on VectorE
    su/sv/st accumulate over the 64 j-blocks on TensorE (stationary = omega column)
No cross-core communication: the host sums 8 tiny partials (the "all-reduce").
"""

from contextlib import ExitStack

import numpy as np

import concourse.bacc as bacc
import concourse.bass as bass
import concourse.tile as tile
from concourse import mybir
from concourse.bass_utils import run_bass_kernel_spmd

N = 8192
NCORES = 8
P = 128
LAM = 0.1

F32 = mybir.dt.float32
BF16 = mybir.dt.bfloat16

# generation dtype for a/b/ab tiles and the matmul stationary weights
GEN_DT = F32
GEN_NP = np.float32


def build_program(n=N, ncores=NCORES, gen_dt=GEN_DT):
    rows = n // ncores            # i-range owned by this core
    nblk = n // P                 # j blocks of 128
    chunk = min(512, rows)        # matmul moving free-dim limit (psum bank)
    nchunk = rows // chunk
    sl = rows // P                # free dim of the bce slice tile

    # single merged input tensor: one DMA -> one semaphore lane, because the
    # ACT instruction encoding only supports a single sync-wait (walrus
    # "too many sync wait commands" with per-tensor DMAs)
    # layout: [obc | ebc | nob | neb | wst(f32) | osl | lsl | wsl]
    W = 2 * rows + 3 * nblk + 3 * sl
    OFF_OBC, OFF_EBC = 0, rows
    OFF_NOB = 2 * rows
    OFF_NEB = OFF_NOB + nblk
    OFF_WST = OFF_NEB + nblk
    OFF_BSL = OFF_WST + nblk

    # Bacc (not plain Bass): its compile() runs generate_event_semaphores,
    # which legalizes multi-semaphore waits — walrus codegen only accepts one
    # sync wait per compute instruction ("Too many sync wait commands")
    nc = bacc.Bacc(None)
    inp = nc.dram_tensor("inp", [P, W], F32, kind="ExternalInput")
    suvt = nc.dram_tensor("suvt", [1, 3 * rows], F32, kind="ExternalOutput")
    bco = nc.dram_tensor("bco", [P, 1], F32, kind="ExternalOutput")

    with tile.TileContext(nc) as tc, ExitStack() as ctx:
        const = ctx.enter_context(tc.tile_pool(name="const", bufs=1))
        work = ctx.enter_context(tc.tile_pool(name="work", bufs=3))
        ps = ctx.enter_context(tc.tile_pool(name="ps", bufs=1, space="PSUM"))
        outp = ctx.enter_context(tc.tile_pool(name="outp", bufs=1))

        inpt = const.tile([P, W], F32, tag="inpt")
        nc.sync.dma_start(out=inpt, in_=inp[:])
        obc = inpt[:, OFF_OBC : OFF_OBC + rows]
        ebc = inpt[:, OFF_EBC : OFF_EBC + rows]
        nobt = inpt[:, OFF_NOB : OFF_NOB + nblk]
        nebt = inpt[:, OFF_NEB : OFF_NEB + nblk]

        # stationary weights through a DVE copy/cast: the first matmul then
        # waits on compute semaphores only (ACT for a, DVE covers wstt+b)
        wstt = const.tile([P, nblk], gen_dt, tag="wstt")
        nc.vector.tensor_copy(out=wstt, in_=inpt[:, OFF_WST : OFF_WST + nblk])

        ups = [ps.tile([1, chunk], F32, name=f"u{c}", tag=f"u{c}") for c in range(nchunk)]
        vps = [ps.tile([1, chunk], F32, name=f"v{c}", tag=f"v{c}") for c in range(nchunk)]
        tps = [ps.tile([1, chunk], F32, name=f"t{c}", tag=f"t{c}") for c in range(nchunk)]

        for k in range(nblk):
            a = work.tile([P, rows], gen_dt, tag="a")
            b = work.tile([P, rows], gen_dt, tag="b")
            ab = work.tile([P, rows], gen_dt, tag="ab")
            # a[j, i] = |o_i - o_j|
            nc.scalar.activation(
                out=a,
                in_=obc,
                func=mybir.ActivationFunctionType.Abs,
                bias=nobt[:, k : k + 1],
                scale=1.0,
            )
            # b[j, i] = |e_i - e_j| (abs_max on DVE tensor_scalar is not a
            # valid HW ALU op — walrus ISA check — so ACT does both gens)
            nc.scalar.activation(
                out=b,
                in_=ebc,
                func=mybir.ActivationFunctionType.Abs,
                bias=nebt[:, k : k + 1],
                scale=1.0,
            )
            nc.vector.tensor_mul(out=ab, in0=a, in1=b)
            first, last = k == 0, k == nblk - 1
            for c in range(nchunk):
                s = bass.ts(c, chunk)
                nc.tensor.matmul(ups[c][:], wstt[:, k : k + 1], a[:, s], start=first, stop=last)
                nc.tensor.matmul(vps[c][:], wstt[:, k : k + 1], b[:, s], start=first, stop=last)
                nc.tensor.matmul(tps[c][:], wstt[:, k : k + 1], ab[:, s], start=first, stop=last)

        uo = outp.tile([1, 3 * rows], F32, tag="uo")
        for grp, base in ((ups, 0), (vps, rows), (tps, 2 * rows)):
            for c in range(nchunk):
                nc.vector.tensor_copy(
                    out=uo[:, base + c * chunk : base + (c + 1) * chunk], in_=grp[c][:]
                )
        nc.sync.dma_start(out=suvt[:], in_=uo)

        # BCE partial over this core's slice: sum((softplus(o) - o*l) * w)
        ot = inpt[:, OFF_BSL : OFF_BSL + sl]
        lt = inpt[:, OFF_BSL + sl : OFF_BSL + 2 * sl]
        wt = inpt[:, OFF_BSL + 2 * sl : OFF_BSL + 3 * sl]
        # softplus(x) = relu(x) + ln(1 + exp(-|x|))  (numerically stable)
        sp = outp.tile([P, sl], F32, tag="sp")
        ol = outp.tile([P, sl], F32, tag="ol")
        tmp = outp.tile([P, sl], F32, tag="tmp")
        nc.scalar.activation(out=tmp, in_=ot, func=mybir.ActivationFunctionType.Abs)
        nc.scalar.activation(
            out=tmp, in_=tmp, func=mybir.ActivationFunctionType.Exp, scale=-1.0
        )
        nc.scalar.activation(
            out=tmp, in_=tmp, func=mybir.ActivationFunctionType.Ln, bias=1.0
        )
        nc.scalar.activation(out=sp, in_=ot, func=mybir.ActivationFunctionType.Relu)
        nc.vector.tensor_add(out=sp, in0=sp, in1=tmp)
        nc.vector.tensor_mul(out=ol, in0=ot, in1=lt)
        nc.vector.tensor_sub(out=sp, in0=sp, in1=ol)
        nc.vector.tensor_mul(out=sp, in0=sp, in1=wt)
        br = outp.tile([P, 1], F32, tag="br")
        nc.vector.reduce_sum(out=br, in_=sp, axis=mybir.AxisListType.X)
        nc.sync.dma_start(out=bco[:], in_=br)

    nc.finalize()
    return nc


def make_in_maps(o, l, e, w, om, n=N, ncores=NCORES):
    rows = n // ncores
    nblk = n // P
    sl = rows // P
    nob = (-o).reshape(nblk, P).T
    neb = (-e).reshape(nblk, P).T
    wstm = om.astype(np.float32).reshape(nblk, P).T
    in_maps = []
    for c in range(ncores):
        r = slice(c * rows, (c + 1) * rows)
        inp = np.concatenate(
            [
                np.broadcast_to(o[r], (P, rows)),
                np.broadcast_to(e[r], (P, rows)),
                nob,
                neb,
                wstm,
                o[r].reshape(sl, P).T,
                l[r].reshape(sl, P).T,
                w[r].reshape(sl, P).T,
            ],
            axis=1,
        )
        in_maps.append({"inp": np.ascontiguousarray(inp, dtype=np.float32)})
    return in_maps


def combine(results, o, e, om, n=N, ncores=NCORES):
    """Host-side O(N) finish: gather per-core partials, apply the exact
    decomposition in float64, return (bce_mean, disco, tot) as float32."""
    rows = n // ncores
    su = np.concatenate([results[c]["suvt"][0, :rows] for c in range(ncores)]).astype(np.float64)
    sv = np.concatenate(
        [results[c]["suvt"][0, rows : 2 * rows] for c in range(ncores)]
    ).astype(np.float64)
    st = np.concatenate(
        [results[c]["suvt"][0, 2 * rows : 3 * rows] for c in range(ncores)]
    ).astype(np.float64)
    bce_sum = float(sum(results[c]["bco"].astype(np.float64).sum() for c in range(ncores)))

    omd = om.astype(np.float64)
    od = o.astype(np.float64)
    ed = e.astype(np.float64)
    nf = float(n)
    S = omd.sum()
    u = su / nf
    v = sv / nf
    T_ab = (omd * st).sum()
    P_uv = (omd * u * v).sum()
    P_uu = (omd * u * u).sum()
    P_vv = (omd * v * v).sum()
    mA = (omd * u).sum() / nf
    mB = (omd * v).sum() / nf
    T_aa = 2.0 * S * (omd * od * od).sum() - 2.0 * (omd * od).sum() ** 2
    T_bb = 2.0 * S * (omd * ed * ed).sum() - 2.0 * (omd * ed).sum() ** 2
    c1 = 2.0 * S - 4.0 * nf
    c2 = 4.0 * nf * nf - 4.0 * nf * S + S * S
    num = (T_ab + c1 * P_uv + c2 * mA * mB) / nf**2
    denA = (T_aa + c1 * P_uu + c2 * mA * mA) / nf**2
    denB = (T_bb + c1 * P_vv + c2 * mB * mB) / nf**2
    disco = num / np.sqrt(denA * denB)
    bce_mean = bce_sum / nf
    tot = bce_mean + LAM * disco
    return (np.float32(bce_mean), np.float32(disco), np.float32(tot))


def run(outputs, labels, event, weights, **spmd_kwargs):
    o = np.asarray(outputs, dtype=np.float32)
    l = np.asarray(labels, dtype=np.float32)
    e = np.asarray(event, dtype=np.float32)
    w = np.asarray(weights, dtype=np.float32)
    assert o.shape == (N,)

    # normalized weights, mimicking the reference's f32 computation
    nw = (w * np.float32(N) / w.sum(dtype=np.float32)).astype(np.float32)
    om = nw.astype(GEN_NP)

    nc = build_program()
    in_maps = make_in_maps(o, l, e, w, om)
    bkr = run_bass_kernel_spmd(nc, in_maps, list(range(NCORES)), **spmd_kwargs)
    return combine(bkr.results, o, e, om), bkr


def kernel(outputs, labels, event, weights):
    out, _ = run(outputs, labels, event, weights)
    return out


# revision 23
# speedup vs baseline: 2.0896x; 2.0896x over previous
"""Trainium2 Bass kernel for BCE + distance-decorrelation (DisCo) loss.

Reference math (N = 8192):
    bce  = mean((softplus(o) - o*l) * w)
    nw   = w * N / sum(w)
    a_ij = |o_i - o_j|, b_ij = |e_i - e_j|
    u_i  = (1/N) sum_j a_ij nw_j          (amatavg)
    A    = a - u_j - u_i + mA,  mA = (1/N) sum nw u    (same for B with v, mB)
    num  = (1/N^2) sum_ij nw_i nw_j A_ij B_ij
    den  = [(1/N^2) sum nw nw A^2] [(1/N^2) sum nw nw B^2]
    disco = num / sqrt(den);  tot = bce + 0.1 * disco

Exact algebraic decomposition (for arbitrary device weights omega, S = sum omega):
    num*N^2  = T_ab + (2S-4N) P_uv + (4N^2-4NS+S^2) mA mB
    denA*N^2 = T_aa + (2S-4N) P_uu + (4N^2-4NS+S^2) mA^2
    T_aa     = 2 S sum(om o^2) - 2 (sum om o)^2          (closed form, O(N))
    T_ab     = sum_i om_i t_i,  t_i = sum_j om_j a_ij b_ij
so the only O(N^2) device work is three per-row weighted sums:
    su_i = sum_j om_j a_ij, sv_i = sum_j om_j b_ij, st_i = sum_j om_j a_ij b_ij.

Sharding: core c owns rows [c*1024, (c+1)*1024). Tiles are laid out
[j (partition, 128 per block, 64 blocks) x i (free, 1024 rows)]:
    a = Abs(o_bcast + (-o_j))            on ScalarE (activation, per-partition bias)
    b = abs_max(e_bcast + (-e_j), 0)     on VectorE (one tensor_scalar op)
    ab = a * b                           on VectorE
    su/sv/st accumulate over the 64 j-blocks on TensorE (stationary = omega column)
No cross-core communication: the host sums 8 tiny partials (the "all-reduce").
"""

from contextlib import ExitStack

import numpy as np

import concourse.bacc as bacc
import concourse.bass as bass
import concourse.tile as tile
from concourse import mybir
from concourse.bass_utils import run_bass_kernel_spmd

N = 8192
NCORES = 8
P = 128
LAM = 0.1

F32 = mybir.dt.float32
BF16 = mybir.dt.bfloat16

# generation dtype for a/b/ab tiles and the matmul stationary weights.
# fp16 everywhere: full-rate PE matmuls (fp32 lowers to 2 HW passes), DVE
# 2x/4x perf modes, and ~3e-3 relative error on disco (validated offline —
# bf16 products would give ~1e-2..2e-1 due to the ~5000x cancellation in num)
GEN_DT = mybir.dt.float16
GEN_NP = np.float16


def build_program(n=N, ncores=NCORES, gen_dt=GEN_DT):
    rows = n // ncores            # i-range owned by this core
    nblk = n // P                 # j blocks of 128
    chunk = min(512, rows)        # matmul moving free-dim limit (psum bank)
    nchunk = rows // chunk
    sl = rows // P                # free dim of the bce slice tile

    # single merged input tensor: one DMA -> one semaphore lane, because the
    # ACT instruction encoding only supports a single sync-wait (walrus
    # "too many sync wait commands" with per-tensor DMAs)
    # layout: [obc | ebc | nob | neb | wst(f32) | osl | lsl | wsl]
    W = 2 * rows + 3 * nblk + 3 * sl
    OFF_OBC, OFF_EBC = 0, rows
    OFF_NOB = 2 * rows
    OFF_NEB = OFF_NOB + nblk
    OFF_WST = OFF_NEB + nblk
    OFF_BSL = OFF_WST + nblk

    # Bacc (not plain Bass): its compile() runs generate_event_semaphores,
    # which legalizes multi-semaphore waits — walrus codegen only accepts one
    # sync wait per compute instruction ("Too many sync wait commands")
    nc = bacc.Bacc(None)
    inp = nc.dram_tensor("inp", [P, W], F32, kind="ExternalInput")
    suvt = nc.dram_tensor("suvt", [1, 3 * rows], F32, kind="ExternalOutput")
    bco = nc.dram_tensor("bco", [P, 1], F32, kind="ExternalOutput")

    with tile.TileContext(nc) as tc, ExitStack() as ctx:
        const = ctx.enter_context(tc.tile_pool(name="const", bufs=1))
        work = ctx.enter_context(tc.tile_pool(name="work", bufs=3))
        ps = ctx.enter_context(tc.tile_pool(name="ps", bufs=1, space="PSUM"))
        outp = ctx.enter_context(tc.tile_pool(name="outp", bufs=1))

        inpt = const.tile([P, W], F32, tag="inpt")
        nc.sync.dma_start(out=inpt, in_=inp[:])
        obc = inpt[:, OFF_OBC : OFF_OBC + rows]
        ebc = inpt[:, OFF_EBC : OFF_EBC + rows]
        nobt = inpt[:, OFF_NOB : OFF_NOB + nblk]
        nebt = inpt[:, OFF_NEB : OFF_NEB + nblk]

        # stationary weights through a DVE copy/cast: the first matmul then
        # waits on compute semaphores only (ACT for a, DVE covers wstt+b)
        wstt = const.tile([P, nblk], gen_dt, tag="wstt")
        nc.vector.tensor_copy(out=wstt, in_=inpt[:, OFF_WST : OFF_WST + nblk])

        ups = [ps.tile([1, chunk], F32, name=f"u{c}", tag=f"u{c}") for c in range(nchunk)]
        vps = [ps.tile([1, chunk], F32, name=f"v{c}", tag=f"v{c}") for c in range(nchunk)]
        tps = [ps.tile([1, chunk], F32, name=f"t{c}", tag=f"t{c}") for c in range(nchunk)]

        for k in range(nblk):
            a = work.tile([P, rows], gen_dt, tag="a")
            b = work.tile([P, rows], gen_dt, tag="b")
            ab = work.tile([P, rows], gen_dt, tag="ab")
            # a[j, i] = |o_i - o_j|
            nc.scalar.activation(
                out=a,
                in_=obc,
                func=mybir.ActivationFunctionType.Abs,
                bias=nobt[:, k : k + 1],
                scale=1.0,
            )
            # b[j, i] = |e_i - e_j| on DVE: subtract (fp32 2x mode), then
            # clear the sign bit on the int32 view (abs; abs_max/fused forms
            # are not valid HW TensorScalar ops)
            nc.vector.tensor_scalar(
                out=b,
                in0=ebc,
                scalar1=nebt[:, k : k + 1],
                scalar2=None,
                op0=mybir.AluOpType.add,
            )
            bi = b.bitcast(mybir.dt.int16)
            nc.vector.tensor_scalar(
                out=bi,
                in0=bi,
                scalar1=0x7FFF,
                scalar2=None,
                op0=mybir.AluOpType.bitwise_and,
            )
            # product split across DVE and GpSimd to balance engine load
            half = rows // 2
            nc.vector.tensor_mul(
                out=ab[:, 0:half], in0=a[:, 0:half], in1=b[:, 0:half]
            )
            nc.gpsimd.tensor_tensor(
                out=ab[:, half:rows],
                in0=a[:, half:rows],
                in1=b[:, half:rows],
                op=mybir.AluOpType.mult,
            )
            first, last = k == 0, k == nblk - 1
            wk = wstt[:, k : k + 1]
            for c in range(nchunk):
                s = bass.ts(c, chunk)
                nc.tensor.matmul(ups[c][:], wk, a[:, s], start=first, stop=last)
                nc.tensor.matmul(vps[c][:], wk, b[:, s], start=first, stop=last)
                nc.tensor.matmul(tps[c][:], wk, ab[:, s], start=first, stop=last)

        uo = outp.tile([1, 3 * rows], F32, tag="uo")
        for grp, base in ((ups, 0), (vps, rows), (tps, 2 * rows)):
            for c in range(nchunk):
                nc.vector.tensor_copy(
                    out=uo[:, base + c * chunk : base + (c + 1) * chunk], in_=grp[c][:]
                )
        nc.sync.dma_start(out=suvt[:], in_=uo)

        # BCE partial over this core's slice: sum((softplus(o) - o*l) * w)
        ot = inpt[:, OFF_BSL : OFF_BSL + sl]
        lt = inpt[:, OFF_BSL + sl : OFF_BSL + 2 * sl]
        wt = inpt[:, OFF_BSL + 2 * sl : OFF_BSL + 3 * sl]
        # softplus(x) = relu(x) + ln(1 + exp(-|x|))  (numerically stable)
        sp = outp.tile([P, sl], F32, tag="sp")
        ol = outp.tile([P, sl], F32, tag="ol")
        tmp = outp.tile([P, sl], F32, tag="tmp")
        nc.scalar.activation(out=tmp, in_=ot, func=mybir.ActivationFunctionType.Abs)
        nc.scalar.activation(
            out=tmp, in_=tmp, func=mybir.ActivationFunctionType.Exp, scale=-1.0
        )
        nc.scalar.activation(
            out=tmp, in_=tmp, func=mybir.ActivationFunctionType.Ln, bias=1.0
        )
        nc.scalar.activation(out=sp, in_=ot, func=mybir.ActivationFunctionType.Relu)
        nc.vector.tensor_add(out=sp, in0=sp, in1=tmp)
        nc.vector.tensor_mul(out=ol, in0=ot, in1=lt)
        nc.vector.tensor_sub(out=sp, in0=sp, in1=ol)
        nc.vector.tensor_mul(out=sp, in0=sp, in1=wt)
        br = outp.tile([P, 1], F32, tag="br")
        nc.vector.reduce_sum(out=br, in_=sp, axis=mybir.AxisListType.X)
        nc.sync.dma_start(out=bco[:], in_=br)

    nc.finalize()
    return nc


def make_in_maps(o, l, e, w, om, n=N, ncores=NCORES):
    rows = n // ncores
    nblk = n // P
    sl = rows // P
    nob = (-o).reshape(nblk, P).T
    neb = (-e).reshape(nblk, P).T
    wstm = om.astype(np.float32).reshape(nblk, P).T
    in_maps = []
    for c in range(ncores):
        r = slice(c * rows, (c + 1) * rows)
        inp = np.concatenate(
            [
                np.broadcast_to(o[r], (P, rows)),
                np.broadcast_to(e[r], (P, rows)),
                nob,
                neb,
                wstm,
                o[r].reshape(sl, P).T,
                l[r].reshape(sl, P).T,
                w[r].reshape(sl, P).T,
            ],
            axis=1,
        )
        in_maps.append({"inp": np.ascontiguousarray(inp, dtype=np.float32)})
    return in_maps


def combine(results, o, e, om, n=N, ncores=NCORES):
    """Host-side O(N) finish: gather per-core partials, apply the exact
    decomposition in float64, return (bce_mean, disco, tot) as float32."""
    rows = n // ncores
    su = np.concatenate([results[c]["suvt"][0, :rows] for c in range(ncores)]).astype(np.float64)
    sv = np.concatenate(
        [results[c]["suvt"][0, rows : 2 * rows] for c in range(ncores)]
    ).astype(np.float64)
    st = np.concatenate(
        [results[c]["suvt"][0, 2 * rows : 3 * rows] for c in range(ncores)]
    ).astype(np.float64)
    bce_sum = float(sum(results[c]["bco"].astype(np.float64).sum() for c in range(ncores)))

    omd = om.astype(np.float64)
    od = o.astype(np.float64)
    ed = e.astype(np.float64)
    nf = float(n)
    S = omd.sum()
    u = su / nf
    v = sv / nf
    T_ab = (omd * st).sum()
    P_uv = (omd * u * v).sum()
    P_uu = (omd * u * u).sum()
    P_vv = (omd * v * v).sum()
    mA = (omd * u).sum() / nf
    mB = (omd * v).sum() / nf
    T_aa = 2.0 * S * (omd * od * od).sum() - 2.0 * (omd * od).sum() ** 2
    T_bb = 2.0 * S * (omd * ed * ed).sum() - 2.0 * (omd * ed).sum() ** 2
    c1 = 2.0 * S - 4.0 * nf
    c2 = 4.0 * nf * nf - 4.0 * nf * S + S * S
    num = (T_ab + c1 * P_uv + c2 * mA * mB) / nf**2
    denA = (T_aa + c1 * P_uu + c2 * mA * mA) / nf**2
    denB = (T_bb + c1 * P_vv + c2 * mB * mB) / nf**2
    disco = num / np.sqrt(denA * denB)
    bce_mean = bce_sum / nf
    tot = bce_mean + LAM * disco
    return (np.float32(bce_mean), np.float32(disco), np.float32(tot))


def run(outputs, labels, event, weights, **spmd_kwargs):
    o = np.asarray(outputs, dtype=np.float32)
    l = np.asarray(labels, dtype=np.float32)
    e = np.asarray(event, dtype=np.float32)
    w = np.asarray(weights, dtype=np.float32)
    assert o.shape == (N,)

    # normalized weights, mimicking the reference's f32 computation
    nw = (w * np.float32(N) / w.sum(dtype=np.float32)).astype(np.float32)
    om = nw.astype(GEN_NP)

    nc = build_program()
    in_maps = make_in_maps(o, l, e, w, om)
    bkr = run_bass_kernel_spmd(nc, in_maps, list(range(NCORES)), **spmd_kwargs)
    return combine(bkr.results, o, e, om), bkr


def kernel(outputs, labels, event, weights):
    out, _ = run(outputs, labels, event, weights)
    return out


# revision 27
# speedup vs baseline: 2.4798x; 1.1867x over previous
"""Trainium2 Bass kernel for BCE + distance-decorrelation (DisCo) loss.

Reference math (N = 8192):
    bce  = mean((softplus(o) - o*l) * w)
    nw   = w * N / sum(w)
    a_ij = |o_i - o_j|, b_ij = |e_i - e_j|
    u_i  = (1/N) sum_j a_ij nw_j          (amatavg)
    A    = a - u_j - u_i + mA,  mA = (1/N) sum nw u    (same for B with v, mB)
    num  = (1/N^2) sum_ij nw_i nw_j A_ij B_ij
    den  = [(1/N^2) sum nw nw A^2] [(1/N^2) sum nw nw B^2]
    disco = num / sqrt(den);  tot = bce + 0.1 * disco

Exact algebraic decomposition (for arbitrary device weights omega, S = sum omega):
    num*N^2  = T_ab + (2S-4N) P_uv + (4N^2-4NS+S^2) mA mB
    denA*N^2 = T_aa + (2S-4N) P_uu + (4N^2-4NS+S^2) mA^2
    T_aa     = 2 S sum(om o^2) - 2 (sum om o)^2          (closed form, O(N))
    T_ab     = sum_i om_i t_i,  t_i = sum_j om_j a_ij b_ij
so the only O(N^2) device work is three per-row weighted sums:
    su_i = sum_j om_j a_ij, sv_i = sum_j om_j b_ij, st_i = sum_j om_j a_ij b_ij.

Sharding: core c owns rows [c*1024, (c+1)*1024). Tiles are laid out
[j (partition, 128 per block, 64 blocks) x i (free, 1024 rows)]:
    a = Abs(o_bcast + (-o_j))            on ScalarE (activation, per-partition bias)
    b = abs_max(e_bcast + (-e_j), 0)     on VectorE (one tensor_scalar op)
    ab = a * b                           on VectorE
    su/sv/st accumulate over the 64 j-blocks on TensorE (stationary = omega column)
No cross-core communication: the host sums 8 tiny partials (the "all-reduce").
"""

from contextlib import ExitStack

import numpy as np

import concourse.bacc as bacc
import concourse.bass as bass
import concourse.tile as tile
from concourse import mybir
from concourse.bass_utils import run_bass_kernel_spmd

N = 8192
NCORES = 8
P = 128
LAM = 0.1

F32 = mybir.dt.float32
BF16 = mybir.dt.bfloat16

# dtypes: a/b in bf16 (generation rounding cancels through the exact identity
# as long as u/v are computed from the same rounded values), product in fp16
# (bf16 products would give ~1e-2..2e-1 error due to the ~5000x cancellation
# in num; fp16 keeps it at ~2e-3). omega is rounded to bf16 on the host so its
# bf16 and fp16 stationary copies are bit-identical (bf16 normals are exactly
# representable in fp16), keeping one omega across all three matmuls.
import ml_dtypes

GEN_DT = mybir.dt.bfloat16
PROD_DT = mybir.dt.float16
GEN_NP = ml_dtypes.bfloat16


def build_program(n=N, ncores=NCORES, gen_dt=GEN_DT):
    rows = n // ncores            # i-range owned by this core
    nblk = n // P                 # j blocks of 128
    chunk = min(512, rows)        # matmul moving free-dim limit (psum bank)
    nchunk = rows // chunk
    sl = rows // P                # free dim of the bce slice tile

    # single merged input tensor: one DMA -> one semaphore lane, because the
    # ACT instruction encoding only supports a single sync-wait (walrus
    # "too many sync wait commands" with per-tensor DMAs)
    # layout: [obc | ebc | nob | neb | wst(f32) | osl | lsl | wsl]
    W = 2 * rows + 3 * nblk + 3 * sl
    OFF_OBC, OFF_EBC = 0, rows
    OFF_NOB = 2 * rows
    OFF_NEB = OFF_NOB + nblk
    OFF_WST = OFF_NEB + nblk
    OFF_BSL = OFF_WST + nblk

    # Bacc (not plain Bass): its compile() runs generate_event_semaphores,
    # which legalizes multi-semaphore waits — walrus codegen only accepts one
    # sync wait per compute instruction ("Too many sync wait commands")
    nc = bacc.Bacc(None)
    inp = nc.dram_tensor("inp", [P, W], F32, kind="ExternalInput")
    suvt = nc.dram_tensor("suvt", [1, 3 * rows], F32, kind="ExternalOutput")
    bco = nc.dram_tensor("bco", [P, 1], F32, kind="ExternalOutput")

    with tile.TileContext(nc) as tc, ExitStack() as ctx:
        const = ctx.enter_context(tc.tile_pool(name="const", bufs=1))
        work = ctx.enter_context(tc.tile_pool(name="work", bufs=3))
        ps = ctx.enter_context(tc.tile_pool(name="ps", bufs=1, space="PSUM"))
        outp = ctx.enter_context(tc.tile_pool(name="outp", bufs=1))

        inpt = const.tile([P, W], F32, tag="inpt")
        nc.sync.dma_start(out=inpt, in_=inp[:])
        obc = inpt[:, OFF_OBC : OFF_OBC + rows]
        ebc = inpt[:, OFF_EBC : OFF_EBC + rows]
        nobt = inpt[:, OFF_NOB : OFF_NOB + nblk]
        nebt = inpt[:, OFF_NEB : OFF_NEB + nblk]

        # stationary weights through DVE copies/casts: the first matmul then
        # waits on compute semaphores only. Two bit-identical stationary
        # copies: bf16 for the u/v matmuls, fp16 for the t matmul.
        wstg = const.tile([P, nblk], gen_dt, tag="wstg")
        wstp = const.tile([P, nblk], PROD_DT, tag="wstp")
        nc.vector.tensor_copy(out=wstg, in_=inpt[:, OFF_WST : OFF_WST + nblk])
        nc.vector.tensor_copy(out=wstp, in_=inpt[:, OFF_WST : OFF_WST + nblk])

        # one psum bank per i-chunk; u/v/t accumulate at partition rows
        # 0/32/64 so the three matmuls land on disjoint PE column groups
        # (tile_position col tiling -> they run concurrently)
        banks = [ps.tile([65, chunk], F32, name=f"bk{c}", tag=f"bk{c}") for c in range(nchunk)]

        half = rows // 2
        for k in range(nblk):
            a = work.tile([P, rows], gen_dt, tag="a")
            b = work.tile([P, rows], gen_dt, tag="b")
            ab = work.tile([P, rows], PROD_DT, tag="ab")
            # a[j, i] = |o_i - o_j| on ACT
            nc.scalar.activation(
                out=a,
                in_=obc,
                func=mybir.ActivationFunctionType.Abs,
                bias=nobt[:, k : k + 1],
                scale=1.0,
            )
            # b[j, i] = |e_i - e_j|: DVE subtract (2x mode), then abs — half
            # via sign-bit clear on the int16 view (DVE), half on ACT, to
            # balance the two engines
            nc.vector.tensor_scalar(
                out=b,
                in0=ebc,
                scalar1=nebt[:, k : k + 1],
                scalar2=None,
                op0=mybir.AluOpType.add,
            )
            bi = b.bitcast(mybir.dt.int16)
            nc.vector.tensor_scalar(
                out=bi[:, 0:half],
                in0=bi[:, 0:half],
                scalar1=0x7FFF,
                scalar2=None,
                op0=mybir.AluOpType.bitwise_and,
            )
            nc.scalar.activation(
                out=b[:, half:rows],
                in_=b[:, half:rows],
                func=mybir.ActivationFunctionType.Abs,
            )
            # product bf16*bf16 -> fp16 on DVE (2x mode on 16-bit inputs)
            nc.vector.tensor_mul(out=ab, in0=a, in1=b)
            first, last = k == 0, k == nblk - 1
            wg = wstg[:, k : k + 1]
            wp = wstp[:, k : k + 1]
            for c in range(nchunk):
                s = bass.ts(c, chunk)
                bk = banks[c]
                nc.tensor.matmul(bk[0:1, :], wg, a[:, s], start=first, stop=last,
                                 tile_position=(0, 0), skip_group_check=True)
                nc.tensor.matmul(bk[32:33, :], wg, b[:, s], start=first, stop=last,
                                 tile_position=(0, 32), skip_group_check=True)
                nc.tensor.matmul(bk[64:65, :], wp, ab[:, s], start=first, stop=last,
                                 tile_position=(0, 64), skip_group_check=True)

        uo = outp.tile([1, 3 * rows], F32, tag="uo")
        for row, base in ((0, 0), (32, rows), (64, 2 * rows)):
            for c in range(nchunk):
                nc.vector.tensor_copy(
                    out=uo[:, base + c * chunk : base + (c + 1) * chunk],
                    in_=banks[c][row : row + 1, :],
                )
        nc.sync.dma_start(out=suvt[:], in_=uo)

        # BCE partial over this core's slice: sum((softplus(o) - o*l) * w)
        ot = inpt[:, OFF_BSL : OFF_BSL + sl]
        lt = inpt[:, OFF_BSL + sl : OFF_BSL + 2 * sl]
        wt = inpt[:, OFF_BSL + 2 * sl : OFF_BSL + 3 * sl]
        # softplus(x) = relu(x) + ln(1 + exp(-|x|))  (numerically stable)
        sp = outp.tile([P, sl], F32, tag="sp")
        ol = outp.tile([P, sl], F32, tag="ol")
        tmp = outp.tile([P, sl], F32, tag="tmp")
        nc.scalar.activation(out=tmp, in_=ot, func=mybir.ActivationFunctionType.Abs)
        nc.scalar.activation(
            out=tmp, in_=tmp, func=mybir.ActivationFunctionType.Exp, scale=-1.0
        )
        nc.scalar.activation(
            out=tmp, in_=tmp, func=mybir.ActivationFunctionType.Ln, bias=1.0
        )
        nc.scalar.activation(out=sp, in_=ot, func=mybir.ActivationFunctionType.Relu)
        nc.vector.tensor_add(out=sp, in0=sp, in1=tmp)
        nc.vector.tensor_mul(out=ol, in0=ot, in1=lt)
        nc.vector.tensor_sub(out=sp, in0=sp, in1=ol)
        nc.vector.tensor_mul(out=sp, in0=sp, in1=wt)
        br = outp.tile([P, 1], F32, tag="br")
        nc.vector.reduce_sum(out=br, in_=sp, axis=mybir.AxisListType.X)
        nc.sync.dma_start(out=bco[:], in_=br)

    nc.finalize()
    return nc


def make_in_maps(o, l, e, w, om, n=N, ncores=NCORES):
    rows = n // ncores
    nblk = n // P
    sl = rows // P
    nob = (-o).reshape(nblk, P).T
    neb = (-e).reshape(nblk, P).T
    wstm = om.astype(np.float32).reshape(nblk, P).T
    in_maps = []
    for c in range(ncores):
        r = slice(c * rows, (c + 1) * rows)
        inp = np.concatenate(
            [
                np.broadcast_to(o[r], (P, rows)),
                np.broadcast_to(e[r], (P, rows)),
                nob,
                neb,
                wstm,
                o[r].reshape(sl, P).T,
                l[r].reshape(sl, P).T,
                w[r].reshape(sl, P).T,
            ],
            axis=1,
        )
        in_maps.append({"inp": np.ascontiguousarray(inp, dtype=np.float32)})
    return in_maps


def combine(results, o, e, om, n=N, ncores=NCORES):
    """Host-side O(N) finish: gather per-core partials, apply the exact
    decomposition in float64, return (bce_mean, disco, tot) as float32."""
    rows = n // ncores
    su = np.concatenate([results[c]["suvt"][0, :rows] for c in range(ncores)]).astype(np.float64)
    sv = np.concatenate(
        [results[c]["suvt"][0, rows : 2 * rows] for c in range(ncores)]
    ).astype(np.float64)
    st = np.concatenate(
        [results[c]["suvt"][0, 2 * rows : 3 * rows] for c in range(ncores)]
    ).astype(np.float64)
    bce_sum = float(sum(results[c]["bco"].astype(np.float64).sum() for c in range(ncores)))

    omd = om.astype(np.float64)
    od = o.astype(np.float64)
    ed = e.astype(np.float64)
    nf = float(n)
    S = omd.sum()
    u = su / nf
    v = sv / nf
    T_ab = (omd * st).sum()
    P_uv = (omd * u * v).sum()
    P_uu = (omd * u * u).sum()
    P_vv = (omd * v * v).sum()
    mA = (omd * u).sum() / nf
    mB = (omd * v).sum() / nf
    T_aa = 2.0 * S * (omd * od * od).sum() - 2.0 * (omd * od).sum() ** 2
    T_bb = 2.0 * S * (omd * ed * ed).sum() - 2.0 * (omd * ed).sum() ** 2
    c1 = 2.0 * S - 4.0 * nf
    c2 = 4.0 * nf * nf - 4.0 * nf * S + S * S
    num = (T_ab + c1 * P_uv + c2 * mA * mB) / nf**2
    denA = (T_aa + c1 * P_uu + c2 * mA * mA) / nf**2
    denB = (T_bb + c1 * P_vv + c2 * mB * mB) / nf**2
    disco = num / np.sqrt(denA * denB)
    bce_mean = bce_sum / nf
    tot = bce_mean + LAM * disco
    return (np.float32(bce_mean), np.float32(disco), np.float32(tot))


def run(outputs, labels, event, weights, **spmd_kwargs):
    o = np.asarray(outputs, dtype=np.float32)
    l = np.asarray(labels, dtype=np.float32)
    e = np.asarray(event, dtype=np.float32)
    w = np.asarray(weights, dtype=np.float32)
    assert o.shape == (N,)

    # normalized weights, mimicking the reference's f32 computation
    nw = (w * np.float32(N) / w.sum(dtype=np.float32)).astype(np.float32)
    om = nw.astype(GEN_NP)

    nc = build_program()
    in_maps = make_in_maps(o, l, e, w, om)
    bkr = run_bass_kernel_spmd(nc, in_maps, list(range(NCORES)), **spmd_kwargs)
    return combine(bkr.results, o, e, om), bkr


def kernel(outputs, labels, event, weights):
    out, _ = run(outputs, labels, event, weights)
    return out


# revision 33
# speedup vs baseline: 3.2515x; 1.3112x over previous
"""Trainium2 Bass kernel for BCE + distance-decorrelation (DisCo) loss.

Reference math (N = 8192):
    bce  = mean((softplus(o) - o*l) * w)
    nw   = w * N / sum(w)
    a_ij = |o_i - o_j|, b_ij = |e_i - e_j|
    u_i  = (1/N) sum_j a_ij nw_j          (amatavg)
    A    = a - u_j - u_i + mA,  mA = (1/N) sum nw u    (same for B with v, mB)
    num  = (1/N^2) sum_ij nw_i nw_j A_ij B_ij
    den  = [(1/N^2) sum nw nw A^2] [(1/N^2) sum nw nw B^2]
    disco = num / sqrt(den);  tot = bce + 0.1 * disco

Exact algebraic decomposition (for arbitrary device weights omega, S = sum omega):
    num*N^2  = T_ab + (2S-4N) P_uv + (4N^2-4NS+S^2) mA mB
    denA*N^2 = T_aa + (2S-4N) P_uu + (4N^2-4NS+S^2) mA^2
    T_aa     = 2 S sum(om o^2) - 2 (sum om o)^2          (closed form, O(N))
    T_ab     = sum_i om_i t_i,  t_i = sum_j om_j a_ij b_ij
so the only O(N^2) device work is three per-row weighted sums:
    su_i = sum_j om_j a_ij, sv_i = sum_j om_j b_ij, st_i = sum_j om_j a_ij b_ij.

Sharding: core c owns rows [c*1024, (c+1)*1024). Tiles are laid out
[j (partition, 128 per block, 64 blocks) x i (free, 1024 rows)]:
    a = Abs(o_bcast + (-o_j))            on ScalarE (activation, per-partition bias)
    b = abs_max(e_bcast + (-e_j), 0)     on VectorE (one tensor_scalar op)
    ab = a * b                           on VectorE
    su/sv/st accumulate over the 64 j-blocks on TensorE (stationary = omega column)
No cross-core communication: the host sums 8 tiny partials (the "all-reduce").
"""

from contextlib import ExitStack

import numpy as np

import concourse.bacc as bacc
import concourse.bass as bass
import concourse.tile as tile
from concourse import mybir
from concourse.bass_utils import run_bass_kernel_spmd

N = 8192
NCORES = 8
P = 128
LAM = 0.1

F32 = mybir.dt.float32
BF16 = mybir.dt.bfloat16

# dtypes: a/b in bf16 (generation rounding cancels through the exact identity
# as long as u/v are computed from the same rounded values), product in fp16
# (bf16 products would give ~1e-2..2e-1 error due to the ~5000x cancellation
# in num; fp16 keeps it at ~2e-3). omega is rounded to bf16 on the host so its
# bf16 and fp16 stationary copies are bit-identical (bf16 normals are exactly
# representable in fp16), keeping one omega across all three matmuls.
import ml_dtypes

GEN_DT = mybir.dt.bfloat16
PROD_DT = mybir.dt.float16
GEN_NP = ml_dtypes.bfloat16


def build_program(n=N, ncores=NCORES, gen_dt=GEN_DT):
    rows = n // ncores            # i-range owned by this core
    nblk = n // P                 # j blocks of 128
    chunk = min(512, rows)        # matmul moving free-dim limit (psum bank)
    nchunk = rows // chunk
    sl = rows // P                # free dim of the bce slice tile

    # merged input tensor (f32 container). o/e broadcasts carry bf16-rounded
    # values PACKED in pairs (bitcast views), halving their footprint and
    # unlocking DVE 4x mode for the subtract. All abs-diffs are computed on
    # bf16-rounded o/e on BOTH sides, keeping the matrices exactly symmetric
    # (the decomposition identity needs symmetry).
    # layout: [obc.bf16/2 | ebc.bf16/2 | nob | neb | wst(f32) | osl | lsl | wsl]
    hrows = rows // 2
    W = rows + 3 * nblk + 3 * sl
    OFF_OBC, OFF_EBC = 0, hrows
    OFF_NOB = 2 * hrows
    OFF_NEB = OFF_NOB + nblk
    OFF_WST = OFF_NEB + nblk
    OFF_BSL = OFF_WST + nblk

    # Bacc (not plain Bass): its compile() runs generate_event_semaphores,
    # which legalizes multi-semaphore waits — walrus codegen only accepts one
    # sync wait per compute instruction ("Too many sync wait commands")
    nc = bacc.Bacc(None)
    inp = nc.dram_tensor("inp", [P, W], F32, kind="ExternalInput")
    suvt = nc.dram_tensor("suvt", [1, 3 * rows], F32, kind="ExternalOutput")
    bco = nc.dram_tensor("bco", [P, 1], F32, kind="ExternalOutput")

    with tile.TileContext(nc) as tc, ExitStack() as ctx:
        const = ctx.enter_context(tc.tile_pool(name="const", bufs=1))
        work = ctx.enter_context(tc.tile_pool(name="work", bufs=3))
        ps = ctx.enter_context(tc.tile_pool(name="ps", bufs=1, space="PSUM"))
        outp = ctx.enter_context(tc.tile_pool(name="outp", bufs=1))

        # 4 parallel DMAs on different queues (multi-sem waits are legalized
        # by Bacc's event-semaphore pass, so this is safe now)
        inpt = const.tile([P, W], F32, tag="inpt")
        ndma = 4 if W % 4 == 0 else (2 if W % 2 == 0 else 1)
        qw = W // ndma
        for q in range(ndma):
            nc.sync.dma_start(
                out=inpt[:, q * qw : (q + 1) * qw], in_=inp[:, q * qw : (q + 1) * qw]
            )
        obc = inpt[:, OFF_OBC : OFF_OBC + hrows].bitcast(gen_dt)
        ebc = inpt[:, OFF_EBC : OFF_EBC + hrows].bitcast(gen_dt)
        nobt = inpt[:, OFF_NOB : OFF_NOB + nblk]
        nebt = inpt[:, OFF_NEB : OFF_NEB + nblk]

        # stationary weights through DVE copies/casts: the first matmul then
        # waits on compute semaphores only. Two bit-identical stationary
        # copies: bf16 for the u/v matmuls, fp16 for the t matmul.
        wstg = const.tile([P, nblk], gen_dt, tag="wstg")
        wstp = const.tile([P, nblk], PROD_DT, tag="wstp")
        nc.vector.tensor_copy(out=wstg, in_=inpt[:, OFF_WST : OFF_WST + nblk])
        nc.vector.tensor_copy(out=wstp, in_=inpt[:, OFF_WST : OFF_WST + nblk])

        # BCE partial over this core's slice, emitted BEFORE the main loop so
        # it executes in the DMA/warmup shadow: sum((softplus(o) - o*l) * w)
        ot = inpt[:, OFF_BSL : OFF_BSL + sl]
        lt = inpt[:, OFF_BSL + sl : OFF_BSL + 2 * sl]
        wt = inpt[:, OFF_BSL + 2 * sl : OFF_BSL + 3 * sl]
        # softplus(x) = relu(x) + ln(1 + exp(-|x|))  (numerically stable)
        sp = outp.tile([P, sl], F32, tag="sp")
        ol = outp.tile([P, sl], F32, tag="ol")
        tmp = outp.tile([P, sl], F32, tag="tmp")
        nc.scalar.activation(out=tmp, in_=ot, func=mybir.ActivationFunctionType.Abs)
        nc.scalar.activation(
            out=tmp, in_=tmp, func=mybir.ActivationFunctionType.Exp, scale=-1.0
        )
        nc.scalar.activation(
            out=tmp, in_=tmp, func=mybir.ActivationFunctionType.Ln, bias=1.0
        )
        nc.scalar.activation(out=sp, in_=ot, func=mybir.ActivationFunctionType.Relu)
        nc.vector.tensor_add(out=sp, in0=sp, in1=tmp)
        nc.vector.tensor_mul(out=ol, in0=ot, in1=lt)
        nc.vector.tensor_sub(out=sp, in0=sp, in1=ol)
        nc.vector.tensor_mul(out=sp, in0=sp, in1=wt)
        br = outp.tile([P, 1], F32, tag="br")
        nc.vector.reduce_sum(out=br, in_=sp, axis=mybir.AxisListType.X)
        nc.sync.dma_start(out=bco[:], in_=br)

        # u/v/t accumulators packed 4-per-psum-bank at partition rows
        # 0/32/64/96: matmuls with distinct tile_position col groups run
        # concurrently on disjoint PE sub-arrays
        nq = 3 * nchunk
        nbanks = (nq + 3) // 4
        banks = [
            ps.tile([128, chunk], F32, name=f"bk{bb}", tag=f"bk{bb}")
            for bb in range(nbanks)
        ]
        # quantity order: (u,c0) (v,c0) (t,c0) (u,c1) (v,c1) (t,c1)
        slots = []  # (bank_ap_row, tile_position, which, chunk)
        for idx in range(nq):
            which = idx % 3  # 0=u 1=v 2=t
            c = idx // 3
            bk = banks[idx // 4]
            row = 32 * (idx % 4)
            slots.append((bk[row : row + 1, :], (0, row), which, c))

        for k in range(nblk):
            a = work.tile([P, rows], gen_dt, tag="a")
            b = work.tile([P, rows], gen_dt, tag="b")
            ab = work.tile([P, rows], PROD_DT, tag="ab")
            # a[j, i] = |o_i - o_j| on ACT
            nc.scalar.activation(
                out=a,
                in_=obc,
                func=mybir.ActivationFunctionType.Abs,
                bias=nobt[:, k : k + 1],
                scale=1.0,
            )
            # b[j, i] = |e_i - e_j| on DVE: subtract (4x mode, bf16 in/out),
            # then clear both packed sign bits on the int32 view (2x mode)
            nc.vector.tensor_scalar(
                out=b,
                in0=ebc,
                scalar1=nebt[:, k : k + 1],
                scalar2=None,
                op0=mybir.AluOpType.add,
            )
            bi = b.bitcast(mybir.dt.int32)
            nc.vector.tensor_scalar(
                out=bi,
                in0=bi,
                scalar1=0x7FFF7FFF,
                scalar2=None,
                op0=mybir.AluOpType.bitwise_and,
            )
            # product bf16*bf16 -> fp16 on DVE (2x mode on 16-bit inputs)
            nc.vector.tensor_mul(out=ab, in0=a, in1=b)
            first, last = k == 0, k == nblk - 1
            wg = wstg[:, k : k + 1]
            wp = wstp[:, k : k + 1]
            movs = (a, b, ab)
            wsts = (wg, wg, wp)
            for out_ap, tpos, which, c in slots:
                s = bass.ts(c, chunk)
                nc.tensor.matmul(
                    out_ap,
                    wsts[which],
                    movs[which][:, s],
                    start=first,
                    stop=last,
                    tile_position=tpos,
                    skip_group_check=True,
                )

        uo = outp.tile([1, 3 * rows], F32, tag="uo")
        for out_ap, tpos, which, c in slots:
            base = which * rows + c * chunk
            nc.vector.tensor_copy(out=uo[:, base : base + chunk], in_=out_ap)
        nc.sync.dma_start(out=suvt[:], in_=uo)

    nc.finalize()
    return nc


def make_in_maps(o, l, e, w, om, n=N, ncores=NCORES):
    rows = n // ncores
    nblk = n // P
    sl = rows // P
    # o/e rounded to bf16 on BOTH sides of the abs-diff (broadcast rows AND
    # per-block bias columns) so the distance matrices stay exactly symmetric
    obf = o.astype(GEN_NP)
    ebf = e.astype(GEN_NP)
    of32 = obf.astype(np.float32)
    ef32 = ebf.astype(np.float32)
    nob = (-of32).reshape(nblk, P).T
    neb = (-ef32).reshape(nblk, P).T
    wstm = om.astype(np.float32).reshape(nblk, P).T
    in_maps = []
    for c in range(ncores):
        r = slice(c * rows, (c + 1) * rows)
        # bf16 pairs packed into the f32 container (device bitcasts back)
        opk = np.ascontiguousarray(obf[r]).view(np.float32)
        epk = np.ascontiguousarray(ebf[r]).view(np.float32)
        inp = np.concatenate(
            [
                np.broadcast_to(opk, (P, rows // 2)),
                np.broadcast_to(epk, (P, rows // 2)),
                nob,
                neb,
                wstm,
                o[r].reshape(sl, P).T,
                l[r].reshape(sl, P).T,
                w[r].reshape(sl, P).T,
            ],
            axis=1,
        )
        in_maps.append({"inp": np.ascontiguousarray(inp, dtype=np.float32)})
    return in_maps


def combine(results, o, e, om, n=N, ncores=NCORES):
    """Host-side O(N) finish: gather per-core partials, apply the exact
    decomposition in float64, return (bce_mean, disco, tot) as float32."""
    rows = n // ncores
    su = np.concatenate([results[c]["suvt"][0, :rows] for c in range(ncores)]).astype(np.float64)
    sv = np.concatenate(
        [results[c]["suvt"][0, rows : 2 * rows] for c in range(ncores)]
    ).astype(np.float64)
    st = np.concatenate(
        [results[c]["suvt"][0, 2 * rows : 3 * rows] for c in range(ncores)]
    ).astype(np.float64)
    bce_sum = float(sum(results[c]["bco"].astype(np.float64).sum() for c in range(ncores)))

    omd = om.astype(np.float64)
    od = o.astype(np.float64)
    ed = e.astype(np.float64)
    nf = float(n)
    S = omd.sum()
    u = su / nf
    v = sv / nf
    T_ab = (omd * st).sum()
    P_uv = (omd * u * v).sum()
    P_uu = (omd * u * u).sum()
    P_vv = (omd * v * v).sum()
    mA = (omd * u).sum() / nf
    mB = (omd * v).sum() / nf
    T_aa = 2.0 * S * (omd * od * od).sum() - 2.0 * (omd * od).sum() ** 2
    T_bb = 2.0 * S * (omd * ed * ed).sum() - 2.0 * (omd * ed).sum() ** 2
    c1 = 2.0 * S - 4.0 * nf
    c2 = 4.0 * nf * nf - 4.0 * nf * S + S * S
    num = (T_ab + c1 * P_uv + c2 * mA * mB) / nf**2
    denA = (T_aa + c1 * P_uu + c2 * mA * mA) / nf**2
    denB = (T_bb + c1 * P_vv + c2 * mB * mB) / nf**2
    disco = num / np.sqrt(denA * denB)
    bce_mean = bce_sum / nf
    tot = bce_mean + LAM * disco
    return (np.float32(bce_mean), np.float32(disco), np.float32(tot))


def run(outputs, labels, event, weights, **spmd_kwargs):
    o = np.asarray(outputs, dtype=np.float32)
    l = np.asarray(labels, dtype=np.float32)
    e = np.asarray(event, dtype=np.float32)
    w = np.asarray(weights, dtype=np.float32)
    assert o.shape == (N,)

    # normalized weights, mimicking the reference's f32 computation
    nw = (w * np.float32(N) / w.sum(dtype=np.float32)).astype(np.float32)
    om = nw.astype(GEN_NP)

    nc = build_program()
    in_maps = make_in_maps(o, l, e, w, om)
    bkr = run_bass_kernel_spmd(nc, in_maps, list(range(NCORES)), **spmd_kwargs)
    # the closed-form moments must use the same bf16-rounded o/e the device
    # computed its distance matrices from
    o16 = o.astype(GEN_NP).astype(np.float32)
    e16 = e.astype(GEN_NP).astype(np.float32)
    return combine(bkr.results, o16, e16, om), bkr


def kernel(outputs, labels, event, weights):
    out, _ = run(outputs, labels, event, weights)
    return out


# revision 34
# speedup vs baseline: 3.2843x; 1.0101x over previous
"""Trainium2 Bass kernel for BCE + distance-decorrelation (DisCo) loss.

Reference math (N = 8192):
    bce  = mean((softplus(o) - o*l) * w)
    nw   = w * N / sum(w)
    a_ij = |o_i - o_j|, b_ij = |e_i - e_j|
    u_i  = (1/N) sum_j a_ij nw_j          (amatavg)
    A    = a - u_j - u_i + mA,  mA = (1/N) sum nw u    (same for B with v, mB)
    num  = (1/N^2) sum_ij nw_i nw_j A_ij B_ij
    den  = [(1/N^2) sum nw nw A^2] [(1/N^2) sum nw nw B^2]
    disco = num / sqrt(den);  tot = bce + 0.1 * disco

Exact algebraic decomposition (for arbitrary device weights omega, S = sum omega):
    num*N^2  = T_ab + (2S-4N) P_uv + (4N^2-4NS+S^2) mA mB
    denA*N^2 = T_aa + (2S-4N) P_uu + (4N^2-4NS+S^2) mA^2
    T_aa     = 2 S sum(om o^2) - 2 (sum om o)^2          (closed form, O(N))
    T_ab     = sum_i om_i t_i,  t_i = sum_j om_j a_ij b_ij
so the only O(N^2) device work is three per-row weighted sums:
    su_i = sum_j om_j a_ij, sv_i = sum_j om_j b_ij, st_i = sum_j om_j a_ij b_ij.

Sharding: core c owns rows [c*1024, (c+1)*1024). Tiles are laid out
[j (partition, 128 per block, 64 blocks) x i (free, 1024 rows)]:
    a = Abs(o_bcast + (-o_j))            on ScalarE (activation, per-partition bias)
    b = abs_max(e_bcast + (-e_j), 0)     on VectorE (one tensor_scalar op)
    ab = a * b                           on VectorE
    su/sv/st accumulate over the 64 j-blocks on TensorE (stationary = omega column)
No cross-core communication: the host sums 8 tiny partials (the "all-reduce").
"""

from contextlib import ExitStack

import numpy as np

import concourse.bacc as bacc
import concourse.bass as bass
import concourse.tile as tile
from concourse import mybir
from concourse.bass_utils import run_bass_kernel_spmd

N = 8192
NCORES = 8
P = 128
LAM = 0.1

F32 = mybir.dt.float32
BF16 = mybir.dt.bfloat16

# dtypes: a/b in bf16 (generation rounding cancels through the exact identity
# as long as u/v are computed from the same rounded values), product in fp16
# (bf16 products would give ~1e-2..2e-1 error due to the ~5000x cancellation
# in num; fp16 keeps it at ~2e-3). omega is rounded to bf16 on the host so its
# bf16 and fp16 stationary copies are bit-identical (bf16 normals are exactly
# representable in fp16), keeping one omega across all three matmuls.
import ml_dtypes

GEN_DT = mybir.dt.bfloat16
PROD_DT = mybir.dt.float16
GEN_NP = ml_dtypes.bfloat16


def build_program(n=N, ncores=NCORES, gen_dt=GEN_DT):
    rows = n // ncores            # i-range owned by this core
    nblk = n // P                 # j blocks of 128
    chunk = min(512, rows)        # matmul moving free-dim limit (psum bank)
    nchunk = rows // chunk
    sl = rows // P                # free dim of the bce slice tile

    # merged input tensor (f32 container). o/e broadcasts carry bf16-rounded
    # values PACKED in pairs (bitcast views), halving their footprint and
    # unlocking DVE 4x mode for the subtract. All abs-diffs are computed on
    # bf16-rounded o/e on BOTH sides, keeping the matrices exactly symmetric
    # (the decomposition identity needs symmetry).
    # layout: [obc.bf16/2 | ebc.bf16/2 | nob | neb | wst(f32) | osl | lsl | wsl]
    hrows = rows // 2
    W = rows + 3 * nblk + 3 * sl
    OFF_OBC, OFF_EBC = 0, hrows
    OFF_NOB = 2 * hrows
    OFF_NEB = OFF_NOB + nblk
    OFF_WST = OFF_NEB + nblk
    OFF_BSL = OFF_WST + nblk

    # Bacc (not plain Bass): its compile() runs generate_event_semaphores,
    # which legalizes multi-semaphore waits — walrus codegen only accepts one
    # sync wait per compute instruction ("Too many sync wait commands")
    nc = bacc.Bacc(None)
    inp = nc.dram_tensor("inp", [P, W], F32, kind="ExternalInput")
    suvt = nc.dram_tensor("suvt", [1, 3 * rows], F32, kind="ExternalOutput")
    bco = nc.dram_tensor("bco", [P, 1], F32, kind="ExternalOutput")

    with tile.TileContext(nc) as tc, ExitStack() as ctx:
        const = ctx.enter_context(tc.tile_pool(name="const", bufs=1))
        work = ctx.enter_context(tc.tile_pool(name="work", bufs=6))
        ps = ctx.enter_context(tc.tile_pool(name="ps", bufs=1, space="PSUM"))
        outp = ctx.enter_context(tc.tile_pool(name="outp", bufs=1))

        # 4 parallel DMAs on different queues (multi-sem waits are legalized
        # by Bacc's event-semaphore pass, so this is safe now)
        inpt = const.tile([P, W], F32, tag="inpt")
        ndma = 4 if W % 4 == 0 else (2 if W % 2 == 0 else 1)
        qw = W // ndma
        for q in range(ndma):
            nc.sync.dma_start(
                out=inpt[:, q * qw : (q + 1) * qw], in_=inp[:, q * qw : (q + 1) * qw]
            )
        obc = inpt[:, OFF_OBC : OFF_OBC + hrows].bitcast(gen_dt)
        ebc = inpt[:, OFF_EBC : OFF_EBC + hrows].bitcast(gen_dt)
        nobt = inpt[:, OFF_NOB : OFF_NOB + nblk]
        nebt = inpt[:, OFF_NEB : OFF_NEB + nblk]

        # stationary weights through DVE copies/casts: the first matmul then
        # waits on compute semaphores only. Two bit-identical stationary
        # copies: bf16 for the u/v matmuls, fp16 for the t matmul.
        wstg = const.tile([P, nblk], gen_dt, tag="wstg")
        wstp = const.tile([P, nblk], PROD_DT, tag="wstp")
        nc.vector.tensor_copy(out=wstg, in_=inpt[:, OFF_WST : OFF_WST + nblk])
        nc.vector.tensor_copy(out=wstp, in_=inpt[:, OFF_WST : OFF_WST + nblk])

        # BCE partial over this core's slice, emitted BEFORE the main loop so
        # it executes in the DMA/warmup shadow: sum((softplus(o) - o*l) * w)
        ot = inpt[:, OFF_BSL : OFF_BSL + sl]
        lt = inpt[:, OFF_BSL + sl : OFF_BSL + 2 * sl]
        wt = inpt[:, OFF_BSL + 2 * sl : OFF_BSL + 3 * sl]
        # softplus(x) = relu(x) + ln(1 + exp(-|x|))  (numerically stable)
        sp = outp.tile([P, sl], F32, tag="sp")
        ol = outp.tile([P, sl], F32, tag="ol")
        tmp = outp.tile([P, sl], F32, tag="tmp")
        nc.scalar.activation(out=tmp, in_=ot, func=mybir.ActivationFunctionType.Abs)
        nc.scalar.activation(
            out=tmp, in_=tmp, func=mybir.ActivationFunctionType.Exp, scale=-1.0
        )
        nc.scalar.activation(
            out=tmp, in_=tmp, func=mybir.ActivationFunctionType.Ln, bias=1.0
        )
        nc.scalar.activation(out=sp, in_=ot, func=mybir.ActivationFunctionType.Relu)
        nc.vector.tensor_add(out=sp, in0=sp, in1=tmp)
        nc.vector.tensor_mul(out=ol, in0=ot, in1=lt)
        nc.vector.tensor_sub(out=sp, in0=sp, in1=ol)
        nc.vector.tensor_mul(out=sp, in0=sp, in1=wt)
        br = outp.tile([P, 1], F32, tag="br")
        nc.vector.reduce_sum(out=br, in_=sp, axis=mybir.AxisListType.X)
        nc.sync.dma_start(out=bco[:], in_=br)

        # u/v/t accumulators packed 4-per-psum-bank at partition rows
        # 0/32/64/96: matmuls with distinct tile_position col groups run
        # concurrently on disjoint PE sub-arrays
        nq = 3 * nchunk
        nbanks = (nq + 3) // 4
        banks = [
            ps.tile([128, chunk], F32, name=f"bk{bb}", tag=f"bk{bb}")
            for bb in range(nbanks)
        ]
        # quantity order: (u,c0) (v,c0) (t,c0) (u,c1) (v,c1) (t,c1)
        slots = []  # (bank_ap_row, tile_position, which, chunk)
        for idx in range(nq):
            which = idx % 3  # 0=u 1=v 2=t
            c = idx // 3
            bk = banks[idx // 4]
            row = 32 * (idx % 4)
            slots.append((bk[row : row + 1, :], (0, row), which, c))

        for k in range(nblk):
            a = work.tile([P, rows], gen_dt, tag="a")
            b = work.tile([P, rows], gen_dt, tag="b")
            ab = work.tile([P, rows], PROD_DT, tag="ab")
            # a[j, i] = |o_i - o_j| on ACT
            nc.scalar.activation(
                out=a,
                in_=obc,
                func=mybir.ActivationFunctionType.Abs,
                bias=nobt[:, k : k + 1],
                scale=1.0,
            )
            # b[j, i] = |e_i - e_j| on DVE: subtract (4x mode, bf16 in/out),
            # then clear both packed sign bits on the int32 view (2x mode)
            nc.vector.tensor_scalar(
                out=b,
                in0=ebc,
                scalar1=nebt[:, k : k + 1],
                scalar2=None,
                op0=mybir.AluOpType.add,
            )
            bi = b.bitcast(mybir.dt.int32)
            nc.vector.tensor_scalar(
                out=bi,
                in0=bi,
                scalar1=0x7FFF7FFF,
                scalar2=None,
                op0=mybir.AluOpType.bitwise_and,
            )
            # product bf16*bf16 -> fp16 on DVE (2x mode on 16-bit inputs)
            nc.vector.tensor_mul(out=ab, in0=a, in1=b)
            first, last = k == 0, k == nblk - 1
            wg = wstg[:, k : k + 1]
            wp = wstp[:, k : k + 1]
            movs = (a, b, ab)
            wsts = (wg, wg, wp)
            for out_ap, tpos, which, c in slots:
                s = bass.ts(c, chunk)
                nc.tensor.matmul(
                    out_ap,
                    wsts[which],
                    movs[which][:, s],
                    start=first,
                    stop=last,
                    tile_position=tpos,
                    skip_group_check=True,
                )

        uo = outp.tile([1, 3 * rows], F32, tag="uo")
        for out_ap, tpos, which, c in slots:
            base = which * rows + c * chunk
            nc.vector.tensor_copy(out=uo[:, base : base + chunk], in_=out_ap)
        nc.sync.dma_start(out=suvt[:], in_=uo)

    nc.finalize()
    return nc


def make_in_maps(o, l, e, w, om, n=N, ncores=NCORES):
    rows = n // ncores
    nblk = n // P
    sl = rows // P
    # o/e rounded to bf16 on BOTH sides of the abs-diff (broadcast rows AND
    # per-block bias columns) so the distance matrices stay exactly symmetric
    obf = o.astype(GEN_NP)
    ebf = e.astype(GEN_NP)
    of32 = obf.astype(np.float32)
    ef32 = ebf.astype(np.float32)
    nob = (-of32).reshape(nblk, P).T
    neb = (-ef32).reshape(nblk, P).T
    wstm = om.astype(np.float32).reshape(nblk, P).T
    in_maps = []
    for c in range(ncores):
        r = slice(c * rows, (c + 1) * rows)
        # bf16 pairs packed into the f32 container (device bitcasts back)
        opk = np.ascontiguousarray(obf[r]).view(np.float32)
        epk = np.ascontiguousarray(ebf[r]).view(np.float32)
        inp = np.concatenate(
            [
                np.broadcast_to(opk, (P, rows // 2)),
                np.broadcast_to(epk, (P, rows // 2)),
                nob,
                neb,
                wstm,
                o[r].reshape(sl, P).T,
                l[r].reshape(sl, P).T,
                w[r].reshape(sl, P).T,
            ],
            axis=1,
        )
        in_maps.append({"inp": np.ascontiguousarray(inp, dtype=np.float32)})
    return in_maps


def combine(results, o, e, om, n=N, ncores=NCORES):
    """Host-side O(N) finish: gather per-core partials, apply the exact
    decomposition in float64, return (bce_mean, disco, tot) as float32."""
    rows = n // ncores
    su = np.concatenate([results[c]["suvt"][0, :rows] for c in range(ncores)]).astype(np.float64)
    sv = np.concatenate(
        [results[c]["suvt"][0, rows : 2 * rows] for c in range(ncores)]
    ).astype(np.float64)
    st = np.concatenate(
        [results[c]["suvt"][0, 2 * rows : 3 * rows] for c in range(ncores)]
    ).astype(np.float64)
    bce_sum = float(sum(results[c]["bco"].astype(np.float64).sum() for c in range(ncores)))

    omd = om.astype(np.float64)
    od = o.astype(np.float64)
    ed = e.astype(np.float64)
    nf = float(n)
    S = omd.sum()
    u = su / nf
    v = sv / nf
    T_ab = (omd * st).sum()
    P_uv = (omd * u * v).sum()
    P_uu = (omd * u * u).sum()
    P_vv = (omd * v * v).sum()
    mA = (omd * u).sum() / nf
    mB = (omd * v).sum() / nf
    T_aa = 2.0 * S * (omd * od * od).sum() - 2.0 * (omd * od).sum() ** 2
    T_bb = 2.0 * S * (omd * ed * ed).sum() - 2.0 * (omd * ed).sum() ** 2
    c1 = 2.0 * S - 4.0 * nf
    c2 = 4.0 * nf * nf - 4.0 * nf * S + S * S
    num = (T_ab + c1 * P_uv + c2 * mA * mB) / nf**2
    denA = (T_aa + c1 * P_uu + c2 * mA * mA) / nf**2
    denB = (T_bb + c1 * P_vv + c2 * mB * mB) / nf**2
    disco = num / np.sqrt(denA * denB)
    bce_mean = bce_sum / nf
    tot = bce_mean + LAM * disco
    return (np.float32(bce_mean), np.float32(disco), np.float32(tot))


def run(outputs, labels, event, weights, **spmd_kwargs):
    o = np.asarray(outputs, dtype=np.float32)
    l = np.asarray(labels, dtype=np.float32)
    e = np.asarray(event, dtype=np.float32)
    w = np.asarray(weights, dtype=np.float32)
    assert o.shape == (N,)

    # normalized weights, mimicking the reference's f32 computation
    nw = (w * np.float32(N) / w.sum(dtype=np.float32)).astype(np.float32)
    om = nw.astype(GEN_NP)

    nc = build_program()
    in_maps = make_in_maps(o, l, e, w, om)
    bkr = run_bass_kernel_spmd(nc, in_maps, list(range(NCORES)), **spmd_kwargs)
    # the closed-form moments must use the same bf16-rounded o/e the device
    # computed its distance matrices from
    o16 = o.astype(GEN_NP).astype(np.float32)
    e16 = e.astype(GEN_NP).astype(np.float32)
    return combine(bkr.results, o16, e16, om), bkr


def kernel(outputs, labels, event, weights):
    out, _ = run(outputs, labels, event, weights)
    return out
